# revision 11
# baseline (speedup 1.0000x reference)
"""Trainium2 Bass kernel for BasicRecurrentEntityEncoder.

Math (per batch b, entity k, step t):
  enc[b,t,:]  = sum_l mask[b,t,l] * emb[prgrph[b,t,l]] * posmask[l,:]
  g           = sigmoid((h+keys)·s) * sent_mask          (mask folded into gate)
  h_tilda     = sigmoid(h@U + keys@V + s@W)
  h           = normalize(h + g*h_tilda)                  (exact when g=0: h is 0 or unit)

Sharding: data-parallel over batch, 8 paragraphs per core.

Phase 1 uses InstDMAGatherAnt (dma_gather): indices are int16, so the
50000-row table is addressed as two slices (rows 0..32767 and 32768..49999);
every word issues one gather in each slice (the wrong-slice gather fetches a
dummy row that the mask-ones matmul multiplies by zero). 8 rounds of 4096
words; each round: 2 gathers -> DVE posmask multiply -> 64 accumulating
4-col matmuls -> one ACT copy into encT.

Per-core on-chip layouts (BL=8 local paragraphs, K=64, D=128 -> 512 state cols):
  feature-major: col c = b*64 + k, tiles [D=128, 512]     (for PE matmuls)
  layout-B:      chunk j = c>>7, partition p = c&127      (for per-(b,k) scalar ops)
                 so b = 2j + (p>>6), k = p&63

Scan step engines: PE does U/V/W matmuls (float32r), gate row-dots,
transposes; ACT does sigmoids + psum->sbuf copy; DVE does gate select, the
gated update, squared-norm, and an rsqrt via int32-domain magic seed + 1
Newton iteration (ACT Rsqrt is banned and lives in a different
activation-table set than Sigmoid anyway; tensor_tensor_reduce miscomputes
on HW so norm uses square + free-dim reduce).
"""
import numpy as np

import concourse.bass as bass
import concourse.bacc as bacc
import concourse.tile as tile
from concourse import library_config, mybir
from concourse.bass_utils import run_bass_kernel_spmd

F32 = mybir.dt.float32
F32R = mybir.dt.float32r
I16 = mybir.dt.int16
I32 = mybir.dt.int32
AF = mybir.ActivationFunctionType
ALU = mybir.AluOpType

B, T, L, D, K, V = 64, 128, 32, 128, 64, 50000
NCORES = 8
BL = B // NCORES              # 8 paragraphs per core
COLS = BL * K                 # 512 state columns per core
NJ = COLS // 128              # 4 layout-B chunks
WORDS = BL * T * L            # 32768 gathered words per core
NR = 8                        # gather rounds
RW = WORDS // NR              # 4096 words per round
RC = RW // 128                # 32 chunks per round
VSPLIT = 32768                # rows addressable by int16 gather indices
MAGIC = 0x5F3759DF

_cache = {}


def _r(ap):
    return ap.bitcast(F32R)


def _build_nc():
    nc = bacc.Bacc(None, target_bir_lowering=False)

    emb_t = nc.dram_tensor("emb", [V, D], F32, kind="ExternalInput")
    gixA_t = nc.dram_tensor("gixA", [128, NR * (RW // 16)], I16, kind="ExternalInput")
    gixB_t = nc.dram_tensor("gixB", [128, NR * (RW // 16)], I16, kind="ExternalInput")
    moA_t = nc.dram_tensor("moA", [128, NR * RC * 4], F32R, kind="ExternalInput")
    moB_t = nc.dram_tensor("moB", [128, NR * RC * 4], F32R, kind="ExternalInput")
    posrep_t = nc.dram_tensor("posrep", [128, 128], F32, kind="ExternalInput")
    keysT_t = nc.dram_tensor("keysT", [128, COLS], F32R, kind="ExternalInput")
    U_t = nc.dram_tensor("Uw", [D, D], F32R, kind="ExternalInput")
    V_t = nc.dram_tensor("Vw", [D, D], F32R, kind="ExternalInput")
    W_t = nc.dram_tensor("Ww", [D, D], F32R, kind="ExternalInput")
    mscal_t = nc.dram_tensor("maskscal", [128, 4 * T], F32, kind="ExternalInput")
    oh_t = nc.dram_tensor("onehot32", [128, 32], F32, kind="ExternalInput")
    id_t = nc.dram_tensor("ident", [128, 128], F32R, kind="ExternalInput")
    z_t = nc.dram_tensor("zeros", [128, COLS], F32R, kind="ExternalInput")
    out_t = nc.dram_tensor("h_out", [BL, K, D], F32, kind="ExternalOutput")

    RSEG = RW // 16           # 256 idx columns per round

    with tile.TileContext(nc) as tc:
        with tc.tile_pool(name="persist", bufs=1) as pp:
            posrep = pp.tile([128, 128], F32)
            keysT = pp.tile([128, COLS], F32R)
            Uw = pp.tile([D, D], F32R)
            Vw = pp.tile([D, D], F32R)
            Ww = pp.tile([D, D], F32R)
            mscal = pp.tile([128, 4 * T], F32)      # [p, 4t+j] sentence mask
            oh32 = pp.tile([128, 32], F32)
            ident = pp.tile([128, 128], F32R)
            encT = pp.tile([128, T * BL], F32R)      # [d, t*8+b]
            ksst = pp.tile([128, 4 * T], F32)       # [p, 4t+j]
            gixA = pp.tile([128, NR * RSEG], I16)
            gixB = pp.tile([128, NR * RSEG], I16)
            moA = pp.tile([128, NR * RC * 4], F32R)
            moB = pp.tile([128, NR * RC * 4], F32R)
            nc.sync.dma_start(out=posrep, in_=posrep_t[:, :])
            nc.sync.dma_start(out=keysT, in_=keysT_t[:, :])
            nc.sync.dma_start(out=Uw, in_=U_t[:, :])
            nc.sync.dma_start(out=Vw, in_=V_t[:, :])
            nc.sync.dma_start(out=Ww, in_=W_t[:, :])
            nc.sync.dma_start(out=mscal, in_=mscal_t[:, :])
            nc.sync.dma_start(out=oh32, in_=oh_t[:, :])
            nc.sync.dma_start(out=ident, in_=id_t[:, :])
            nc.sync.dma_start(out=gixA, in_=gixA_t[:, :])
            nc.sync.dma_start(out=gixB, in_=gixB_t[:, :])
            nc.sync.dma_start(out=moA, in_=moA_t[:, :])
            nc.sync.dma_start(out=moB, in_=moB_t[:, :])

            embA = emb_t[0:VSPLIT, :]
            embB = emb_t[VSPLIT:V, :]

            # posrep broadcast over the RC chunks of one round
            pos_bc = bass.AP(tensor=posrep.tensor, offset=posrep.offset,
                             ap=[posrep.ap[0], [0, RC], [1, 128]])

            # ---------------- Phase 1: gather + sentence encoder ----------
            nc.gpsimd.load_library(library_config.mlp)
            with tc.tile_pool(name="p1g", bufs=2) as p1g, \
                 tc.tile_pool(name="p1w", bufs=2) as p1w, \
                 tc.tile_pool(name="p1ps", bufs=2, space="PSUM") as p1ps:
                for r in range(NR):
                    ga = p1g.tile([128, RC, 128], F32, tag="ga")
                    nc.gpsimd.dma_gather(
                        ga, embA, gixA[:, r * RSEG:(r + 1) * RSEG],
                        RW, RW, 128, elem_step=128, single_packet=False)
                    gb = p1g.tile([128, RC, 128], F32, tag="gb")
                    nc.gpsimd.dma_gather(
                        gb, embB, gixB[:, r * RSEG:(r + 1) * RSEG],
                        RW, RW, 128, elem_step=128, single_packet=False)
                    wa = p1w.tile([128, RC, 128], F32R, tag="wa")
                    nc.vector.tensor_tensor(out=wa, in0=ga, in1=pos_bc,
                                            op=ALU.mult)
                    wb = p1w.tile([128, RC, 128], F32R, tag="wb")
                    nc.vector.tensor_tensor(out=wb, in0=gb, in1=pos_bc,
                                            op=ALU.mult)
                    penc = p1ps.tile([128, 128], F32, tag="penc")
                    for c in range(RC):
                        mcol = (r * RC + c) * 4
                        nc.tensor.matmul(
                            out=penc[:, 4 * c:4 * c + 4], lhsT=wa[:, c, :],
                            rhs=moA[:, mcol:mcol + 4],
                            start=True, stop=False)
                        nc.tensor.matmul(
                            out=penc[:, 4 * c:4 * c + 4], lhsT=wb[:, c, :],
                            rhs=moB[:, mcol:mcol + 4],
                            start=False, stop=True)
                    nc.scalar.copy(
                        out=encT[:, r * 128:(r + 1) * 128], in_=penc)

            # ---------------- Phase 1.5: ks table -------------------------
            # ks[b,k,t] = sum_d keys[b,k,d]*enc[b,t,d], stored [p, 4t+j]
            with tc.tile_pool(name="ksps", bufs=2, space="PSUM") as ksps:
                for b in range(BL):
                    psk = ksps.tile([64, 128], F32, tag="psk")
                    encb = bass.AP(tensor=encT.tensor, offset=encT.offset + b,
                                   ap=[encT.ap[0], [BL, T]])
                    nc.tensor.matmul(out=psk,
                                     lhsT=keysT[:, b * 64:(b + 1) * 64],
                                     rhs=encb, start=True, stop=True)
                    nc.vector.tensor_copy(
                        out=ksst[(b & 1) * 64:(b & 1) * 64 + 64, (b >> 1)::4],
                        in_=psk)

            # ---------------- Phase 2: the scan ---------------------------
            with tc.tile_pool(name="st", bufs=2) as stp, \
                 tc.tile_pool(name="sm", bufs=3) as smp, \
                 tc.tile_pool(name="scr", bufs=2) as scrp, \
                 tc.tile_pool(name="psA", bufs=2, space="PSUM") as psA, \
                 tc.tile_pool(name="psB", bufs=2, space="PSUM") as psB, \
                 tc.tile_pool(name="psG", bufs=2, space="PSUM") as psG, \
                 tc.tile_pool(name="psH", bufs=2, space="PSUM") as psH:
                hT = stp.tile([128, COLS], F32R, tag="hT")
                hB = stp.tile([128, COLS], F32R, tag="hB")
                nc.sync.dma_start(out=hT, in_=z_t[:, :])
                nc.sync.dma_start(out=hB, in_=z_t[:, :])

                for t in range(T):
                    s_sl = encT[:, 8 * t:8 * t + 8]
                    # gate row-dots: pG[:, 8j+b'] = sum_d hT[d, 128j+p]*s[d,b']
                    pG = psG.tile([128, 32], F32, tag="pG")
                    for j in range(NJ):
                        nc.tensor.matmul(out=pG[:, 8 * j:8 * j + 8],
                                         lhsT=hT[:, 128 * j:128 * (j + 1)],
                                         rhs=s_sl, start=True, stop=True)
                    # pre-activation: U.T@hT + V.T@keysT + W.T@bcast(s)
                    pA = psA.tile([128, COLS], F32, tag="pA")
                    nc.tensor.matmul(out=pA, lhsT=Uw, rhs=hT,
                                     start=True, stop=False)
                    nc.tensor.matmul(out=pA, lhsT=Vw, rhs=keysT,
                                     start=False, stop=False)
                    s_bc = bass.AP(tensor=encT.tensor,
                                   offset=encT.offset + 8 * t,
                                   ap=[encT.ap[0], [1, BL], [0, K]])
                    nc.tensor.matmul(out=pA, lhsT=Ww, rhs=s_bc,
                                     start=False, stop=True)
                    htT = scrp.tile([128, COLS], F32R, tag="htT")
                    nc.scalar.activation(out=htT, in_=pA, func=AF.Sigmoid)

                    # gate: g = sigmoid(dot + ks) * sent_mask   (layout B)
                    gsel = smp.tile([128, 32], F32, tag="gsel")
                    nc.vector.tensor_tensor(out=gsel, in0=pG, in1=oh32,
                                            op=ALU.mult)
                    graw = smp.tile([128, 4], F32, tag="graw")
                    nc.vector.tensor_reduce(
                        out=graw, in_=gsel.rearrange("p (a b) -> p a b", b=8),
                        axis=mybir.AxisListType.X, op=ALU.add)
                    gks = smp.tile([128, 4], F32, tag="gks")
                    nc.vector.tensor_tensor(out=gks, in0=graw,
                                            in1=ksst[:, 4 * t:4 * t + 4],
                                            op=ALU.add)
                    gs = smp.tile([128, 4], F32, tag="gs")
                    nc.scalar.activation(out=gs, in_=gks, func=AF.Sigmoid)
                    gm = smp.tile([128, 4], F32, tag="gm")
                    nc.vector.tensor_tensor(out=gm, in0=gs,
                                            in1=mscal[:, 4 * t:4 * t + 4],
                                            op=ALU.mult)

                    # transpose h_tilda into layout-B
                    pB = psB.tile([128, COLS], F32, tag="pB")
                    for j in range(NJ):
                        nc.tensor.transpose(out=_r(pB[:, 128 * j:128 * (j + 1)]),
                                            in_=htT[:, 128 * j:128 * (j + 1)],
                                            identity=ident)

                    # hn = h + g*h_tilda  (layout B); gm broadcast over d
                    gm_bc = bass.AP(tensor=gm.tensor, offset=gm.offset,
                                    ap=[gm.ap[0], [1, NJ], [0, 128]])
                    ghB = scrp.tile([128, COLS], F32, tag="ghB")
                    nc.vector.tensor_tensor(out=ghB, in0=pB, in1=gm_bc,
                                            op=ALU.mult)
                    hnB = scrp.tile([128, COLS], F32, tag="hnB")
                    nc.vector.tensor_tensor(out=hnB, in0=ghB, in1=hB.bitcast(F32),
                                            op=ALU.add)
                    # ss = sum_d hn^2  (tensor_tensor_reduce miscomputes on HW;
                    # use square + free-dim reduce instead)
                    sq = scrp.tile([128, COLS], F32, tag="sq")
                    nc.vector.tensor_tensor(out=sq, in0=hnB, in1=hnB,
                                            op=ALU.mult)
                    ss = smp.tile([128, 4], F32, tag="ss")
                    nc.vector.tensor_reduce(
                        out=ss, in_=sq.rearrange("p (a b) -> p a b", b=128),
                        axis=mybir.AxisListType.X, op=ALU.add)
                    ssc = smp.tile([128, 4], F32, tag="ssc")
                    nc.vector.tensor_scalar(out=ssc, in0=ss, scalar1=1e-12,
                                            scalar2=None, op0=ALU.max)
                    # inv = rsqrt(ssc): magic seed (int32 value domain) + 1 NR
                    seed = smp.tile([128, 4], I32, tag="seed")
                    nc.vector.tensor_scalar(out=seed, in0=ssc.bitcast(I32),
                                            scalar1=-0.5, scalar2=float(MAGIC),
                                            op0=ALU.mult, op1=ALU.add)
                    y0 = seed.bitcast(F32)
                    t1 = smp.tile([128, 4], F32, tag="t1")
                    t2 = smp.tile([128, 4], F32, tag="t2")
                    t3 = smp.tile([128, 4], F32, tag="t3")
                    inv = smp.tile([128, 4], F32, tag="inv")
                    nc.vector.tensor_tensor(out=t1, in0=y0, in1=y0, op=ALU.mult)
                    nc.vector.tensor_tensor(out=t2, in0=t1, in1=ssc, op=ALU.mult)
                    nc.vector.tensor_scalar(out=t3, in0=t2, scalar1=-0.5,
                                            scalar2=1.5, op0=ALU.mult, op1=ALU.add)
                    nc.vector.tensor_tensor(out=inv, in0=t3, in1=y0, op=ALU.mult)

                    # h' = hn * inv (layout B), then transpose back
                    inv_bc = bass.AP(tensor=inv.tensor, offset=inv.offset,
                                     ap=[inv.ap[0], [1, NJ], [0, 128]])
                    hB_new = stp.tile([128, COLS], F32R, tag="hB")
                    nc.vector.tensor_tensor(out=hB_new, in0=hnB, in1=inv_bc,
                                            op=ALU.mult)
                    pH = psH.tile([128, COLS], F32, tag="pH")
                    for j in range(NJ):
                        nc.tensor.transpose(out=_r(pH[:, 128 * j:128 * (j + 1)]),
                                            in_=hB_new[:, 128 * j:128 * (j + 1)],
                                            identity=ident)
                    hT_new = stp.tile([128, COLS], F32R, tag="hT")
                    nc.scalar.copy(out=hT_new, in_=pH)
                    hB, hT = hB_new, hT_new

                # -------- output: h[b,k,:] = hB[(b&1)*64+k, 128*(b>>1)+:] --
                for b in range(BL):
                    src = hB.bitcast(F32)[(b & 1) * 64:(b & 1) * 64 + 64,
                             128 * (b >> 1):128 * (b >> 1) + 128]
                    nc.sync.dma_start(out=out_t[b, :, :], in_=src)
    nc.compile()
    return nc


def _wrap16(seg):
    # dma_gather index layout: idx i at [i%16, i//16], replicated to 128 parts
    n = seg.shape[0]
    arr = seg.reshape(n // 16, 16).T.astype(np.int16)     # [16, n//16]
    return np.tile(arr, (8, 1))                            # [128, n//16]


def _prep_core(core, prgrph, prgrph_mask, embedding_matrix, positional_mask,
               Uw, Vw, Ww, keys):
    b0 = core * BL
    pr = prgrph[b0:b0 + BL]          # [8, T, L]
    pm = prgrph_mask[b0:b0 + BL]
    ky = keys[b0:b0 + BL]            # [8, K, D]

    vids = np.ascontiguousarray(pr.transpose(1, 0, 2)).reshape(-1)  # (t,b,l)
    low = vids < VSPLIT
    idxA = np.where(low, vids, 0).astype(np.int16)
    idxB = np.where(low, 0, vids - VSPLIT).astype(np.int16)
    gixA = np.concatenate(
        [_wrap16(idxA[r * RW:(r + 1) * RW]) for r in range(NR)], axis=1)
    gixB = np.concatenate(
        [_wrap16(idxB[r * RW:(r + 1) * RW]) for r in range(NR)], axis=1)

    maskf = pm.transpose(1, 0, 2).reshape(-1).astype(np.float32)
    mA = (maskf * low).reshape(-1, 4, 32)        # [chunks, j, 32]
    mB = (maskf * (~low)).reshape(-1, 4, 32)
    nch = mA.shape[0]                             # 256 chunks

    def mo_pack(mw):
        mo = np.zeros((nch, 128, 4), dtype=np.float32)
        for jj in range(4):
            mo[:, jj * 32:(jj + 1) * 32, jj] = mw[:, jj, :]
        # -> [128, chunk*4]
        return np.ascontiguousarray(mo.transpose(1, 0, 2).reshape(128, nch * 4))

    moA = mo_pack(mA)
    moB = mo_pack(mB)

    posrep = np.ascontiguousarray(np.tile(positional_mask, (4, 1))).astype(np.float32)
    keysT = np.ascontiguousarray(ky.transpose(2, 0, 1).reshape(D, COLS))

    # layout-B: partition p, chunk j -> b = 2j + (p>>6)
    p_ar = np.arange(128)
    j_ar = np.arange(4)
    b_of = 2 * j_ar[None, :] + (p_ar[:, None] >> 6)          # [128, 4]
    msent = pm.any(axis=2).astype(np.float32)                # [8, T]
    mscal = np.ascontiguousarray(
        msent[b_of].transpose(0, 2, 1).reshape(128, 4 * T))  # [p, 4t+j]
    oh32 = np.zeros((128, 32), dtype=np.float32)
    for jj in range(4):
        oh32[p_ar, 8 * jj + b_of[:, jj]] = 1.0
    ident = np.eye(128, dtype=np.float32)

    return {
        "emb": np.ascontiguousarray(embedding_matrix.astype(np.float32)),
        "gixA": gixA, "gixB": gixB, "moA": moA, "moB": moB,
        "posrep": posrep, "keysT": keysT,
        "Uw": np.ascontiguousarray(Uw.astype(np.float32)),
        "Vw": np.ascontiguousarray(Vw.astype(np.float32)),
        "Ww": np.ascontiguousarray(Ww.astype(np.float32)),
        "maskscal": mscal, "onehot32": oh32, "ident": ident,
        "zeros": np.zeros((128, COLS), dtype=np.float32),
    }


def kernel(prgrph, prgrph_mask, embedding_matrix, positional_mask,
           Uw, Vw, Ww, keys, _trace=False):
    prgrph = np.asarray(prgrph)
    prgrph_mask = np.asarray(prgrph_mask)
    embedding_matrix = np.asarray(embedding_matrix, dtype=np.float32)
    positional_mask = np.asarray(positional_mask, dtype=np.float32)
    Uw = np.asarray(Uw, dtype=np.float32)
    Vw = np.asarray(Vw, dtype=np.float32)
    Ww = np.asarray(Ww, dtype=np.float32)
    keys = np.asarray(keys, dtype=np.float32)

    if "nc" not in _cache:
        _cache["nc"] = _build_nc()
    nc = _cache["nc"]

    in_maps = [_prep_core(c, prgrph, prgrph_mask, embedding_matrix,
                          positional_mask, Uw, Vw, Ww, keys)
               for c in range(NCORES)]
    res = run_bass_kernel_spmd(nc, in_maps, core_ids=list(range(NCORES)),
                               trace=_trace)
    outs = [np.asarray(r["h_out"]).reshape(BL, K, D) for r in res.results]
    full = np.concatenate(outs, axis=0)
    if _trace:
        kernel.last_results = res
    return full
